# revision 20
# baseline (speedup 1.0000x reference)
"""Trainium2 Bass kernel for nn_BinaryPathEncoder.

Math: for each position p, the ordered product of rotation matrices along
p's binary path (LSB-first, leading 1-bit stripped):
    R(p) = M_{b0} @ M_{b1} @ ... @ M_{b(k-1)},  M_b = expm(B_b - B_b^T)^T
Splitting the <=16-step path into 6+5+5 bit chunks gives
R(p) = R(idxA) @ R(idxB) @ R(idxC) with idxA<128, idxB<64, idxC<64, so two
small fp16 SBUF tables (natural R[q], q<64, and transposed R[q]^T, q<128)
cover every position with 2 matmuls:
  product1: X1T = matmul(lhsT=Rn[idxB](staged), rhs=Rt[idxA]) = (TA@TB)^T
  product2: O   = matmul(lhsT=X1T,              rhs=Rn[idxC]) = TA@TB@TC

v2 changes vs baseline (553us):
 - PE data-dependent addressing via BATCHED multi-register TENSOR_LOADs:
   one 24-value load per 12 positions per operand stream (A and C), with
   host-precomputed {off, off+DIM} pairs -> no reg_alu/snap on the PE NX.
 - gpsimd staging offsets also batch-loaded (16 regs / load).
 - Output DMA batched 4 positions per dma_start with a DRAM layout that
   matches SBUF slot order (host re-gathers).
 - expm: Taylor s=3/n=5 (plenty for fp16 tables), 4 chains (b x {E, E^T})
   interleaved, DVE/ACT split, PSUM-accumulated "+a" term.
"""

import contextlib
import numpy as np

DIM = 256
NCORES = 8
P = 128

NAT_E = 63                     # natural table entries (q in [1,64))
TRA_E = 65                     # transposed entries: slot0=identity, slots 1..64 = q in [64,128)
ENT = 512                      # elements per partition per entry (2 kc x 256)
NAT_STRIDE = NAT_E * ENT
TRA_STRIDE = TRA_E * ENT

NSTAGE = 4                     # psum slots per mm stream (mm1: ps0-3, mm2: ps4-7)
NSTAGE_B = 8                   # lhsT staging slots (absorbs DMA latency)
NSX = 8                        # X1T staging slots
NOUT = 8                       # output buffer slots (2 batches of 4)
OBATCH = 4                     # positions per output DMA
LAG = 4                        # mm2 lags mm1 by LAG positions
RB = 8                         # positions per PE register batch (16 regs/bank)
GB = 16                        # positions per gpsimd register batch
EXPM_S = 3                     # scaling: A = skew / 2^s
EXPM_N = 5                     # Taylor (Horner) order

_NC_CACHE = {}
LAST_RESULTS = None


def _build_nc(npos, debug=False, dump=False):
    from concourse import bass, bacc, mybir

    f32 = mybir.dt.float32
    f32r = mybir.dt.float32r
    f16 = mybir.dt.float16
    i32 = mybir.dt.int32
    Sub = mybir.AluOpType.subtract
    Add = mybir.AluOpType.add
    Eq = mybir.AluOpType.is_equal

    nc = bacc.Bacc("TRN2", target_bir_lowering=False, debug=debug)

    prims_ext = nc.dram_tensor("prims", [2, DIM, DIM], f32, kind="ExternalInput")
    assert npos % OBATCH == 0
    offs_a_ext = nc.dram_tensor("offs_a", [1, 2 * npos], i32, kind="ExternalInput")
    offs_c_ext = nc.dram_tensor("offs_c", [1, 2 * npos], i32, kind="ExternalInput")
    offs_gp_ext = nc.dram_tensor("offs_gp", [1, npos], i32, kind="ExternalInput")
    # [npos/4, P, 4, 2, DIM]: one output DMA per 4 positions, src/dst APs match
    out_ext = nc.dram_tensor("out", [npos // OBATCH, P, OBATCH, 2, DIM], f16,
                             kind="ExternalOutput")

    with contextlib.ExitStack() as ctx:
        sem = {}
        for name in (["in_sem", "pe_sem", "dve_sem", "act_sem", "gps_sem",
                      "mm1_sem", "mm2_sem", "dvex_sem"]
                     + [f"stg_s{j}" for j in range(4)]
                     + [f"dma_s{j}" for j in range(2)]):
            sem[name] = ctx.enter_context(nc.semaphore(name))

        # ---- persistent SBUF ----
        rn = ctx.enter_context(nc.sbuf_tensor("rn", [P, NAT_STRIDE], f16))
        rt = ctx.enter_context(nc.sbuf_tensor("rt", [P, TRA_STRIDE], f16))
        offs_a = ctx.enter_context(nc.sbuf_tensor("offs_a_sb", [1, 2 * npos], i32))
        offs_c = ctx.enter_context(nc.sbuf_tensor("offs_c_sb", [1, 2 * npos], i32))
        offs_gp = ctx.enter_context(nc.sbuf_tensor("offs_gp_sb", [1, npos], i32))
        pbf = ctx.enter_context(nc.sbuf_tensor("pbf", [P, 2, 2, DIM], f16))
        identf = ctx.enter_context(nc.sbuf_tensor("identf", [P, 2, DIM], f32))
        # ---- expm temporaries: scoped; reused by the position-phase staging
        # buffers (runtime lifetimes disjoint) ----
        tmp_ctx = contextlib.ExitStack()
        prim = tmp_ctx.enter_context(nc.sbuf_tensor("prim", [P, 2, 2, DIM], f32))
        # per-b sign buffers: amat[b][0] = -A_b, amat[b][1] = +A_b  (A = skew/2^s)
        amat = tmp_ctx.enter_context(nc.sbuf_tensor("amat", [P, 2, 2, 2, DIM], f32r))
        # 4 Horner/squaring chains: [b][0]=E-chain, [b][1]=ET-chain
        ybuf = tmp_ctx.enter_context(nc.sbuf_tensor("ybuf", [P, 2, 2, 2, DIM], f32r))
        identr = tmp_ctx.enter_context(nc.sbuf_tensor("identr", [P, 2, DIM], f32r))
        ci = tmp_ctx.enter_context(nc.sbuf_tensor("ci", [P, DIM], f32))
        pi = tmp_ctx.enter_context(nc.sbuf_tensor("pi", [P, 2], f32))
        tmp_ctx.close()
        stag_b = ctx.enter_context(nc.sbuf_tensor("stag_b", [P, NSTAGE_B, 2, DIM], f16))
        stag_x = ctx.enter_context(nc.sbuf_tensor("stag_x", [P, NSX, 2, DIM], f16))
        outb = ctx.enter_context(nc.sbuf_tensor("outb", [P, NOUT, 2, DIM], f16))
        # ---- PSUM: 8 banks ----
        ps = [ctx.enter_context(nc.psum_tensor(f"ps{j}", [P, 2, DIM], f32))
              for j in range(8)]

        ident128 = identf[:, 0, 0:P]

        def ent3(tab, q):
            """table entry q as a [P, 2, DIM] static AP"""
            if tab is rn:
                stride, slot = NAT_STRIDE, q - 1
            else:
                stride, slot = TRA_STRIDE, (0 if q == 1 else q - 63)
            return bass.AP(tab, slot * ENT, [[stride, P], [DIM, 2], [1, DIM]])

        cnt = {k: 0 for k in sem}
        entry_done = {}
        pe_prog, dve_prog, act_prog, gps_prog, sync_prog = [], [], [], [], []

        # ---------------- DMA in (sync engine) ----------------
        def s_in(s):
            s.dma_start(offs_a[:, :], offs_a_ext[:, :]).then_inc(sem["in_sem"], 16)
            s.dma_start(offs_c[:, :], offs_c_ext[:, :]).then_inc(sem["in_sem"], 16)
            s.dma_start(offs_gp[:, :], offs_gp_ext[:, :]).then_inc(sem["in_sem"], 16)
            for b in range(2):
                for kc in range(2):
                    s.dma_start(prim[:, b, kc, :],
                                prims_ext[b, kc * P:(kc + 1) * P, :],
                                ).then_inc(sem["in_sem"], 16)
        sync_prog.append(s_in)
        cnt["in_sem"] = 16 * 7
        IN_ALL = cnt["in_sem"]

        # ---------------- identity construction ----------------
        def g_iota(g):
            g.iota(ci[:, :], [[1, DIM]], channel_multiplier=0,
                   allow_small_or_imprecise_dtypes=True)
            g.iota(pi[:, 0:1], [[1, 1]], channel_multiplier=1,
                   allow_small_or_imprecise_dtypes=True)
            g.iota(pi[:, 1:2], [[1, 1]], base=P, channel_multiplier=1,
                   allow_small_or_imprecise_dtypes=True).then_inc(sem["gps_sem"], 1)
        gps_prog.append(g_iota)
        cnt["gps_sem"] += 1

        def d_ident(d, w=cnt["gps_sem"]):
            d.wait_ge(sem["gps_sem"], w)
            for kc in range(2):
                d.tensor_tensor(out=identf[:, kc, :], in0=ci[:, :],
                                in1=pi[:, kc:kc + 1].to_broadcast([P, DIM]), op=Eq)
            d.drain()
            d.tensor_copy(ent3(rn, 1), identf[:, :, :])
            d.tensor_copy(ent3(rt, 1), identf[:, :, :])
            d.tensor_copy(identr[:, :, :],
                          identf[:, :, :]).then_inc(sem["dve_sem"], 1)
        dve_prog.append(d_ident)
        cnt["dve_sem"] += 1
        ident_done = cnt["dve_sem"]

        # ---------------- expm: 4 interleaved chains ----------------
        # chain (b, t): t=0 tracks E_b = expm(A_b/2^s), t=1 tracks its
        # transpose expm(-A_b/2^s).  Squarings: X <- mm(lhsT=XT, rhs=X).
        inv2s = 1.0 / (2.0 ** EXPM_S)

        # B^T for both b into ps0/ps1
        def p_tr(t, win=IN_ALL, wid=ident_done):
            t.wait_ge(sem["in_sem"], win)
            t.wait_ge(sem["dve_sem"], wid)
            last = None
            for b in range(2):
                for kc in range(2):
                    for mc in range(2):
                        last = t.transpose(
                            out=ps[b][:, kc, mc * P:(mc + 1) * P],
                            in_=prim[:, b, mc, kc * P:(kc + 1) * P],
                            identity=ident128)
                if b == 0:
                    last.then_inc(sem["pe_sem"], 1)
            last.then_inc(sem["pe_sem"], 1)
        pe_prog.append(p_tr)
        cnt["pe_sem"] += 2

        # amat[b][0] = (B^T - B)*inv2s = -A_b ; amat[b][1] = +A_b (DVE, both b)
        # ybuf seeds: ybuf[b][0] = +A/N (E-chain), ybuf[b][1] = -A/N (ET-chain)
        def d_an(d, w=cnt["pe_sem"]):
            d.wait_ge(sem["pe_sem"], w)
            for b in range(2):
                d.tensor_tensor(out=amat[:, b, 0, :, :], in0=ps[b][:, :, :],
                                in1=prim[:, b, :, :], op=Sub)
            d.drain()
            for b in range(2):
                d.tensor_scalar_mul(amat[:, b, 1, :, :], amat[:, b, 0, :, :],
                                    -inv2s)
            d.drain()
            for b in range(2):
                d.tensor_scalar_mul(amat[:, b, 0, :, :], amat[:, b, 0, :, :],
                                    inv2s)
            d.drain()
            for b in range(2):
                d.tensor_scalar_mul(ybuf[:, b, 0, :, :], amat[:, b, 1, :, :],
                                    1.0 / EXPM_N)
                d.tensor_scalar_mul(ybuf[:, b, 1, :, :], amat[:, b, 0, :, :],
                                    1.0 / EXPM_N)
            d.drain()
            d.tensor_copy(ci[:, 0:1], ci[:, 0:1]).then_inc(sem["dve_sem"], 1)
        dve_prog.append(d_an)
        cnt["dve_sem"] += 1

        # Horner rounds k = N-1 .. 1:
        #   ps = a@Y + a  (a = +A for t=0 chain, -A for t=1) via PSUM accum:
        #   lhsT = amat[b][tc] (tc=0: -A so lhsT.T=+A; tc=1: +A so lhsT.T=-A)
        #   Y = ps * (1/k);  chain (b,tc) uses psum bank b*2+tc
        for k in range(EXPM_N - 1, 0, -1):
            def p_h(t_, wdve=cnt["dve_sem"], wact=cnt["act_sem"]):
                t_.wait_ge(sem["dve_sem"], wdve)
                if wact:
                    t_.wait_ge(sem["act_sem"], wact)
                for b in range(2):
                    for tc in range(2):
                        bank = b * 2 + tc
                        last = None
                        for mc in range(2):
                            for kc in range(2):
                                t_.matmul(ps[bank][:, mc, :],
                                          amat[:, b, tc, kc, mc * P:(mc + 1) * P],
                                          ybuf[:, b, tc, kc, :],
                                          start=(kc == 0), stop=False)
                            for kc in range(2):
                                last = t_.matmul(
                                    ps[bank][:, mc, :],
                                    amat[:, b, tc, kc, mc * P:(mc + 1) * P],
                                    identr[:, kc, :],
                                    start=False, stop=(kc == 1))
                        last.then_inc(sem["pe_sem"], 1)
            pe_prog.append(p_h)
            cnt["pe_sem"] += 4

            def x_h(d, k=k, w=cnt["pe_sem"]):
                d.wait_ge(sem["pe_sem"], w)
                for b in range(2):
                    for tc in range(2):
                        d.tensor_scalar_mul(ybuf[:, b, tc, :, :],
                                            ps[b * 2 + tc][:, :, :], 1.0 / k)
                d.drain()
                d.nop().then_inc(sem["dve_sem"], 1)
            dve_prog.append(x_h)
            cnt["dve_sem"] += 1

        # Y += I  (P_b = I + T_1): DVE, all 4 chains
        def d_addI(d, wact=cnt["act_sem"]):
            d.wait_ge(sem["act_sem"], wact)
            d.drain()
            for b in range(2):
                for tc in range(2):
                    d.tensor_tensor(out=ybuf[:, b, tc, :, :],
                                    in0=ybuf[:, b, tc, :, :],
                                    in1=identf[:, :, :], op=Add)
            d.drain()
            d.tensor_copy(ci[:, 0:1], ci[:, 0:1]).then_inc(sem["dve_sem"], 1)
        dve_prog.append(d_addI)
        cnt["dve_sem"] += 1

        # Squarings: X <- mm(lhsT=XT, rhs=X); XT <- mm(lhsT=X, rhs=XT)
        for s_i in range(EXPM_S):
            def p_sq(t_, wdve=cnt["dve_sem"], wact=cnt["act_sem"]):
                t_.wait_ge(sem["dve_sem"], wdve)
                t_.wait_ge(sem["act_sem"], wact)
                for b in range(2):
                    for tc in range(2):
                        bank = b * 2 + tc
                        last = None
                        for mc in range(2):
                            for kc in range(2):
                                last = t_.matmul(
                                    ps[bank][:, mc, :],
                                    ybuf[:, b, 1 - tc, kc, mc * P:(mc + 1) * P],
                                    ybuf[:, b, tc, kc, :],
                                    start=(kc == 0), stop=(kc == 1))
                        last.then_inc(sem["pe_sem"], 1)
            pe_prog.append(p_sq)
            cnt["pe_sem"] += 4

            last_sq = (s_i == EXPM_S - 1)

            def x_sq(d, w=cnt["pe_sem"], last_sq=last_sq):
                d.wait_ge(sem["pe_sem"], w)
                for b in range(2):
                    for tc in range(2):
                        dst = ybuf[:, b, tc, :, :]
                        if last_sq and tc == 0:
                            dst = pbf[:, b, :, :]
                        d.tensor_copy(dst, ps[b * 2 + tc][:, :, :])
                d.drain()
                d.nop().then_inc(sem["dve_sem"], 1)
            dve_prog.append(x_sq)
            cnt["dve_sem"] += 1

        pb_dve, pb_act = cnt["dve_sem"], cnt["act_sem"]

        # ---------------- table build ----------------
        build_items = [("n", q) for q in range(2, 64)] + \
                      [("t", q) for q in range(64, 128)]
        bank_owner = {}
        entry_done[("n", 1)] = ("dve_sem", ident_done)
        entry_done[("t", 1)] = ("dve_sem", ident_done)

        for j, (kind, q) in enumerate(build_items):
            bank = j % 8
            b = q & 1
            par = q >> 1

            waits = []
            if j == 0:
                waits.append(("dve_sem", pb_dve))
                waits.append(("act_sem", pb_act))
            waits.append(entry_done[("n", par)])
            if bank in bank_owner:
                waits.append(bank_owner[bank])

            def p_build(t, kind=kind, b=b, par=par, bank=bank,
                        waits=tuple(waits)):
                for s_, c_ in waits:
                    t.wait_ge(sem[s_], c_)
                last = None
                for mc in range(2):
                    for kc in range(2):
                        if kind == "n":
                            lhsT = pbf[:, b, kc, mc * P:(mc + 1) * P]
                            rhs = ent3(rn, par)[:, kc, :]
                        else:
                            lhsT = ent3(rn, par)[:, kc, mc * P:(mc + 1) * P]
                            rhs = pbf[:, b, kc, :]
                        last = t.matmul(ps[bank][:, mc, :], lhsT, rhs,
                                        start=(kc == 0), stop=(kc == 1))
                last.then_inc(sem["pe_sem"], 1)
            pe_prog.append(p_build)
            cnt["pe_sem"] += 1

            ceng = "dve_sem" if j % 2 == 0 else "act_sem"
            prog = dve_prog if j % 2 == 0 else act_prog
            tab = rn if kind == "n" else rt

            def x_copy(e, tab=tab, q=q, bank=bank, w=cnt["pe_sem"], ceng=ceng):
                e.wait_ge(sem["pe_sem"], w)
                if ceng == "dve_sem":
                    e.tensor_copy(ent3(tab, q),
                                  ps[bank][:, :, :]).then_inc(sem[ceng], 1)
                else:
                    e.mul(ent3(tab, q),
                          ps[bank][:, :, :], 1.0).then_inc(sem[ceng], 1)
            prog.append(x_copy)
            cnt[ceng] += 1
            entry_done[(kind, q)] = (ceng, cnt[ceng])
            bank_owner[bank] = (ceng, cnt[ceng])

        build_dve = cnt["dve_sem"]
        build_act = cnt["act_sem"]

        # ---------------- positions ----------------
        if dump:
            skip_positions = True
        else:
            skip_positions = False
        # gpsimd: stage Rn[idxB] into stag_b ring via SWDGE with batched
        # register loads (GB offsets per TENSOR_LOAD).
        def g_pos(g, bd=build_dve, ba=build_act):
            g.wait_ge(sem["in_sem"], IN_ALL)
            g.wait_ge(sem["dve_sem"], bd)
            g.wait_ge(sem["act_sem"], ba)
            regs = [g.alloc_register(f"gb{r}") for r in range(GB)]
            for i in range(npos):
                r = i % GB
                if r == 0:
                    n = min(GB, npos - i)
                    g.reg_load(regs[:n], offs_gp[0:1, i:i + n])
                if i >= NSTAGE_B and i % 4 == 0:
                    g.wait_ge(sem["mm1_sem"], i - NSTAGE_B + 4)
                src = bass.AP(rn, regs[r], [[NAT_STRIDE, P], [DIM, 2], [1, DIM]])
                # per-slot-pair sems: 16-grain DMA incs from in-flight DMAs
                # interleave, so a shared counter would be ambiguous
                g.dma_start(stag_b[:, i % NSTAGE_B, :, :],
                            src).then_inc(sem[f"stg_s{(i % 8) // 2}"], 16)
        if not skip_positions:
            gps_prog.append(g_pos)

        # PE: per position i: mm1(i) [4 MMs], mm2(i-LAG) [4 MMs].
        def p_pos(t, bd=build_dve, ba=build_act):
            t.wait_ge(sem["dve_sem"], bd)
            t.wait_ge(sem["act_sem"], ba)
            # Registers used in matmul APs are single-assignment: walrus
            # materializes (reg*2 + table_base) into a cached temp at first
            # use, so re-loading a register would leave stale addresses.
            # Allocate FRESH handles per batch and free the previous ones.
            a_state = {"regs": [], "vals": [None] * (2 * RB)}
            c_state = {"regs": [], "vals": [None] * (2 * RB)}
            a_vals = a_state["vals"]
            c_vals = c_state["vals"]

            def load_batch(state, offs_sb, base, n, maxv, tag):
                for r in state["regs"]:
                    t.free_register(r)
                regs = [t.alloc_register(f"{tag}{base}_{r}")
                        for r in range(2 * n)]
                t.reg_load(regs, offs_sb[0:1, 2 * base:2 * (base + n)])
                state["regs"] = regs
                for r in range(2 * n):
                    state["vals"][r] = t.snap(regs[r], donate=True,
                                              min_val=0, max_val=maxv)

            def mm1(i):
                # one accumulation group for the whole bank: start clears the
                # BANK (not the addressed region), so per-mc groups interleaved
                # on one bank would wipe the other mc's partial sums
                slot, bslot, r = i % NSTAGE, i % NSTAGE_B, i % RB
                for kc in range(2):
                    rhs = bass.AP(rt, a_vals[2 * r + kc],
                                  [[TRA_STRIDE, P], [1, DIM]])
                    for mc in range(2):
                        ins = t.matmul(ps[slot][:, mc, :],
                                       stag_b[:, bslot, kc, mc * P:(mc + 1) * P],
                                       rhs, start=(kc == 0 and mc == 0),
                                       stop=(kc == 1 and mc == 1),
                                       skip_group_check=True)
                        if mc == 1 and kc == 1:
                            ins.then_inc(sem["mm1_sem"], 1)

            def mm2(j):
                slot, r = j % NSTAGE, j % RB
                for kc in range(2):
                    rhs = bass.AP(rn, c_vals[2 * r + kc],
                                  [[NAT_STRIDE, P], [1, DIM]])
                    for mc in range(2):
                        ins = t.matmul(ps[NSTAGE + slot][:, mc, :],
                                       stag_x[:, j % NSX, kc, mc * P:(mc + 1) * P],
                                       rhs, start=(kc == 0 and mc == 0),
                                       stop=(kc == 1 and mc == 1),
                                       skip_group_check=True)
                        if mc == 1 and kc == 1:
                            ins.then_inc(sem["mm2_sem"], 1)

            for i in range(npos + LAG):
                if i < npos:
                    if i % RB == 0:
                        n = min(RB, npos - i)
                        load_batch(a_state, offs_a, i, n,
                                   TRA_STRIDE - DIM, "pa")
                    if i % 2 == 0:
                        # stag_b for i, i+1 staged (pair sem: 32 per use-round)
                        t.wait_ge(sem[f"stg_s{(i % 8) // 2}"],
                                  32 * (i // 8 + 1))
                        if i + 1 - NSTAGE >= 0:
                            t.wait_ge(sem["dvex_sem"], i + 1 - NSTAGE + 1)
                    mm1(i)
                j = i - LAG
                if j >= 0:
                    if j % RB == 0:
                        n = min(RB, npos - j)
                        load_batch(c_state, offs_c, j, n,
                                   NAT_STRIDE - DIM, "pc")
                    if j % 2 == 0:
                        # X1T(j), X1T(j+1) ready; out psum slot free
                        t.wait_ge(sem["dvex_sem"], min(j + 2, npos))
                        if j + 1 - NSTAGE >= 0:
                            t.wait_ge(sem["act_sem"], ba + j + 1 - NSTAGE + 1)
                    mm2(j)
        if not skip_positions:
            pe_prog.append(p_pos)

        # DVE: X1T copies psum -> stag_x (fp16)
        def d_pos(d):
            for i in range(npos):
                d.wait_ge(sem["mm1_sem"], i + 1)
                if i >= NSX:
                    d.wait_ge(sem["mm2_sem"], i - NSX + 1)
                d.tensor_copy(stag_x[:, i % NSX, :, :],
                              ps[i % NSTAGE][:, :, :]).then_inc(sem["dvex_sem"], 1)
        if not skip_positions:
            dve_prog.append(d_pos)

        # ACT: out copies psum -> outb (fp16)
        def a_pos(a):
            for i in range(npos):
                oslot = i % NOUT
                b = i // OBATCH
                a.wait_ge(sem["mm2_sem"], i + 1)
                if b >= 2 and i % OBATCH == 0:
                    # this slot-half's previous DMA (batch b-2, same sem) done
                    a.wait_ge(sem[f"dma_s{b % 2}"], 16 * (b // 2))
                a.mul(outb[:, oslot, :, :],
                      ps[NSTAGE + i % NSTAGE][:, :, :], 1.0).then_inc(sem["act_sem"], 1)
        if not skip_positions:
            act_prog.append(a_pos)

        if dump:
            def s_dbg(s, bd=build_dve, ba=build_act):
                s.wait_ge(sem["dve_sem"], bd)
                s.wait_ge(sem["act_sem"], ba)
                s.dma_start(out_ext[0, :, 0:2],
                            pbf[:, :, :, :]).then_inc(sem["mm1_sem"], 16)
                for j, q in enumerate((2, 3, 7, 63)):
                    s.dma_start(out_ext[0, :, 2 + j] if j < 2 else
                                out_ext[1, :, j - 2],
                                ent3(rn, q)).then_inc(sem["mm1_sem"], 16)
                for j, q in enumerate((1, 64, 65, 127)):
                    s.dma_start(out_ext[1, :, 2 + j] if j < 2 else
                                out_ext[2, :, j - 2],
                                ent3(rt, q)).then_inc(sem["mm1_sem"], 16)
                s.wait_ge(sem["mm1_sem"], 16 * 9)
            sync_prog.append(s_dbg)

        # sync: output DMA, one per OBATCH positions; alternate sems so each
        # sem has at most one in-flight DMA (16-grain incs would interleave)
        def s_pos(s, ba=build_act):
            nb = npos // OBATCH
            for k in range(nb):
                s.wait_ge(sem["act_sem"], ba + (k + 1) * OBATCH)
                half = (k % 2) * OBATCH
                s.dma_start(out_ext[k], outb[:, half:half + OBATCH, :, :],
                            ).then_inc(sem[f"dma_s{k % 2}"], 16)
            for j in range(2):
                uses = len([k for k in range(nb) if k % 2 == j])
                if uses:
                    s.wait_ge(sem[f"dma_s{j}"], 16 * uses)
        if not skip_positions:
            sync_prog.append(s_pos)

        # ---------------- emit ----------------
        with nc.Block() as block:
            @block.tensor
            def _(tensor):
                for fn in pe_prog:
                    fn(tensor)

            @block.vector
            def _(vector):
                for fn in dve_prog:
                    fn(vector)

            @block.scalar
            def _(scalar):
                for fn in act_prog:
                    fn(scalar)

            @block.gpsimd
            def _(gpsimd):
                for fn in gps_prog:
                    fn(gpsimd)

            @block.sync
            def _(sync):
                for fn in sync_prog:
                    fn(sync)

    return nc


def _host_offsets(u):
    """u: (n,) int64 positions -> (n,5) int32 element offsets
    [oB, oA0, oA1, oC0, oC1]."""
    u = u.astype(np.int64)
    blen = np.zeros_like(u)
    t = u.copy()
    while np.any(t > 0):
        blen = np.where(t > 0, blen + 1, blen)
        t >>= 1
    k = blen - 1  # path length
    tA = np.minimum(k, 6)
    idxA = (1 << tA) + (u & ((1 << tA) - 1))
    tB = np.clip(k - 6, 0, 5)
    idxB = (1 << tB) + ((u >> 6) & ((1 << tB) - 1))
    tC = np.clip(k - 11, 0, 5)
    idxC = (1 << tC) + ((u >> 11) & ((1 << tC) - 1))
    # short paths (p < 64): the whole product is a natural entry -> (1, p, 1)
    short = u < 64
    idxA = np.where(short, 1, idxA)
    idxB = np.where(short, u, idxB)
    assert idxA.max() < 128 and idxB.max() < 64 and idxC.max() < 64
    assert np.all((idxA == 1) | (idxA >= 64))
    oB = (idxB - 1) * ENT
    oA0 = np.where(idxA == 1, 0, (idxA - 63) * ENT)
    oC0 = (idxC - 1) * ENT
    return np.stack([oB, oA0, oA0 + DIM, oC0, oC0 + DIM], axis=1).astype(np.int32)


def kernel(primitives, identity, unique):
    global LAST_RESULTS
    from concourse.bass_utils import run_bass_kernel_spmd

    prims = np.ascontiguousarray(np.asarray(primitives, dtype=np.float32))
    u = np.asarray(unique).astype(np.int64).ravel()
    n = u.shape[0]
    assert n % NCORES == 0
    npos = n // NCORES

    offs5 = _host_offsets(u)  # (n, 5)

    if npos not in _NC_CACHE:
        nc = _build_nc(npos)
        nc.compile()
        _NC_CACHE[npos] = nc
    nc = _NC_CACHE[npos]

    in_maps = []
    for c in range(NCORES):
        sl = offs5[c * npos:(c + 1) * npos]               # (npos, 5)
        a_arr = np.ascontiguousarray(sl[:, 1:3].reshape(1, 2 * npos))
        c_arr = np.ascontiguousarray(sl[:, 3:5].reshape(1, 2 * npos))
        gp_arr = np.ascontiguousarray(sl[:, 0].reshape(1, npos))
        in_maps.append({"prims": prims, "offs_a": a_arr, "offs_c": c_arr,
                        "offs_gp": gp_arr})

    import os
    trace_dir = os.environ.get("KERNEL_TRACE_DIR")
    res = run_bass_kernel_spmd(nc, in_maps, core_ids=list(range(NCORES)),
                               tmpdir=trace_dir)
    LAST_RESULTS = res

    parts = []
    for c in range(NCORES):
        o = np.asarray(res.results[c]["out"])  # (npos/4, P, 4, 2, DIM) f16
        o = o.transpose(0, 2, 3, 1, 4).reshape(npos, DIM, DIM)
        parts.append(o.astype(np.float32))
    out = np.concatenate(parts, axis=0)

    ident = np.asarray(identity, dtype=np.float32)[0]
    if not np.allclose(ident, np.eye(DIM, dtype=np.float32)):
        out = np.einsum("ij,njk->nik", ident, out).astype(np.float32)
    return out
